# revision 3
# baseline (speedup 1.0000x reference)
"""Trainium2 Bass kernel for nn_CustomLoss_19061064859882.

loss = CE(y_pred, y_true) - penalty/N, where the penalty uses
p1 = softmax(y_pred)[:, 0] and per-class weights from the label histogram.

Device computes per-row sumexp over the 128 classes (data-parallel over the
8 cores, fp16 wire). Host does the O(N) bookkeeping in float64 (log of the
row sums, picked-logit gather, label bincount, final scalar).

v2 design: the elementwise exp is split across TWO engines per job:
  - ACT (scalar) runs native EXP on most rows (1 elem/cycle/lane).
  - DVE (vector) runs a Schraudolph fast-exp on the rest in ONE 4x-mode
    tensor_scalar op: i16 = int16(x * 1477.32 + 15301.3); the int16 bit
    pattern reinterpreted as fp16 approximates e^x (rel err ~2.4%, zero-mean
    after calibrating the additive constant; row-sum error ~0.25%, final
    loss error ~2e-4 << 2e-2 tolerance).
The 128->1 row reduction runs as a halving chain (h1,h2,h3 = pairwise adds,
then a 16-wide tensor_reduce into fp32 obuf), distributed between DVE and
GpSimd. GpSimd's jobs are front-loaded because its window closes early.
Input DMAs are issued from both the (idle) PE ring and the SP ring so
descriptor issue isn't serialized; the out-DMA is split in two to overlap
the compute tail.
"""

import sys

import numpy as np

if "/opt/trn_rl_repo" not in sys.path:
    sys.path.insert(0, "/opt/trn_rl_repo")

N_CORES = 8
N = 262144
C = 128  # classes
M = N // N_CORES  # rows per core (32768)
P = 128  # SBUF partitions
ALPHA = 0.5
BETA = 0.5
EPS = 1e-9

# Schraudolph fast-exp constants (fp16 bit trick), calibrated so the MEAN of
# log(row_sum_approx) - lse is ~0 on N(0,1) logits (see numpy calibration).
FE_A = 1477.3197218702985  # 1024 * log2(e)
FE_B = 15301.33  # 1024 * (15 - delta), delta ~= 0.0573

# Job list: (kb = rows per partition, f_dve = rows handled by DVE fast-exp,
# chain = who runs the halving chain: "dve" all-DVE; "gps1" GpS h1 then DVE
# h2+h3+r16; "gps2" GpS h1+h2 then DVE h3+r16). Bases are cumulative.
# GpS-chained jobs come first: GpSimd is the slowest engine and its window
# closes when the DMA stream ends.
F_BIG = 9  # DVE fast-exp rows per big job
JOBS = [
    dict(kb=8, f=0, chain="gps1"),   # small warm-up job
    dict(kb=16, f=0, chain="gps2"),  # medium
    dict(kb=32, f=F_BIG, chain="gps2"),
    dict(kb=32, f=F_BIG, chain="gps1"),
    dict(kb=32, f=F_BIG, chain="gps1"),
    dict(kb=32, f=F_BIG, chain="dve"),
    dict(kb=32, f=F_BIG, chain="dve"),
    dict(kb=32, f=F_BIG, chain="dve"),
    dict(kb=32, f=F_BIG, chain="dve"),
    dict(kb=8, f=8, chain="dve"),    # small tail job, all-DVE (no EXP)
]
_base = 0
for _j in JOBS:
    _j["base"] = _base
    _base += P * _j["kb"]
assert _base == M

BT = 5  # T (input) buffer slots
BE = 5  # E (exp) buffer slots
BH = 3  # H/H2/H3 chain buffer slots

_CACHE: dict = {}


def _build_nc():
    import contextlib

    import concourse.bacc as bacc
    import concourse.mybir as mybir

    f16 = mybir.dt.float16
    i16 = mybir.dt.int16
    f32 = mybir.dt.float32
    Exp = mybir.ActivationFunctionType.Exp
    X = mybir.AxisListType.X
    Add = mybir.AluOpType.add
    Mult = mybir.AluOpType.mult

    nc = bacc.Bacc(
        "TRN2", target_bir_lowering=False, debug=False, num_devices=N_CORES
    )
    y = nc.dram_tensor("y_pred", [M, C], f16, kind="ExternalInput").ap()
    out = nc.dram_tensor("out", [P, M // P], f32, kind="ExternalOutput").ap()

    KBMAX = 32
    T_s = [nc.alloc_sbuf_tensor(f"Tb{i}", [P, KBMAX, C], f16) for i in range(BT)]
    E_s = [nc.alloc_sbuf_tensor(f"Eb{i}", [P, KBMAX, C], f16) for i in range(BE)]
    H_s = [nc.alloc_sbuf_tensor(f"Hb{i}", [P, KBMAX, C // 2], f16) for i in range(BH)]
    H2_s = [nc.alloc_sbuf_tensor(f"H2b{i}", [P, KBMAX, C // 4], f16) for i in range(BH)]
    H3_s = [nc.alloc_sbuf_tensor(f"H3b{i}", [P, KBMAX, C // 8], f16) for i in range(BH)]
    obuf = nc.alloc_sbuf_tensor("obuf", [P, M // P], f32)
    dum = nc.alloc_sbuf_tensor("dum", [P, 1], f16)

    jobs = list(JOBS)
    n = len(jobs)

    # Static bookkeeping: cumulative semaphore targets.
    # s_exp: +1 per EXP (jobs with ACT rows). s_ts: +1 per ts (jobs with f>0).
    # s_g: +1 per GpSimd op. s_v: +1 per DVE chain op (h1/h2/h3, NOT r16).
    # s_r: +1 per r16. dsem[slot]: +16 per DMA into that slot.
    has_exp = [j["kb"] - j["f"] > 0 for j in jobs]
    has_ts = [j["f"] > 0 for j in jobs]
    nexp = np.cumsum(has_exp).tolist()  # nexp[j] = EXPs through job j
    nts = np.cumsum(has_ts).tolist()

    # Per-job op placement for the chain.
    # h1 owner: gps for gps1/gps2 else dve. h2 owner: gps for gps2 else dve.
    h1_gps = [j["chain"] in ("gps1", "gps2") for j in jobs]
    h2_gps = [j["chain"] == "gps2" for j in jobs]

    # Cumulative GpS op count AFTER each job's GpS ops (h1 and maybe h2).
    g_after_h1 = [0] * n
    g_after_h2 = [0] * n
    g = 0
    for j in range(n):
        if h1_gps[j]:
            g += 1
            g_after_h1[j] = g
            if h2_gps[j]:
                g += 1
        g_after_h2[j] = g
    # Cumulative DVE s_v count after each job's DVE chain ops.
    v_after_h1 = [0] * n
    v_after_h2 = [0] * n
    v_after_h3 = [0] * n
    v = 0
    for j in range(n):
        if not h1_gps[j]:
            v += 1
            v_after_h1[j] = v
        if not h2_gps[j]:
            v += 1
            v_after_h2[j] = v
        v += 1  # h3 always DVE
        v_after_h3[j] = v

    cols = []
    col = 0
    for j in jobs:
        cols.append(col)
        col += j["kb"]
    assert col == M // P

    # First out-DMA covers jobs [0, SPLIT), second covers [SPLIT, n).
    SPLIT = 6
    cols_split = cols[SPLIT]

    with contextlib.ExitStack() as stack:
        block = stack.enter_context(nc.Block())
        dsem = [stack.enter_context(nc.semaphore(f"s_dma{i}")) for i in range(BT)]
        s_exp = stack.enter_context(nc.semaphore("s_exp"))
        s_ts = stack.enter_context(nc.semaphore("s_ts"))
        s_g = stack.enter_context(nc.semaphore("s_g"))
        s_v = stack.enter_context(nc.semaphore("s_v"))
        s_r = stack.enter_context(nc.semaphore("s_r"))
        s_out = stack.enter_context(nc.semaphore("s_out"))
        all_sems = dsem + [s_exp, s_ts, s_g, s_v, s_r, s_out]
        sem_nums = sorted(s.num for s in all_sems)

        def dma_in(eng, i):
            j = jobs[i]
            base, kb = j["base"], j["kb"]
            if i >= BT:
                # T slot reuse: job i-BT's consumers (EXP and ts) must be done.
                if has_exp[i - BT]:
                    eng.wait_ge(s_exp, nexp[i - BT])
                if has_ts[i - BT]:
                    eng.wait_ge(s_ts, nts[i - BT])
            yj = y[base : base + P * kb].rearrange("(p k) c -> p k c", p=P)
            eng.dma_start(out=T_s[i % BT].ap()[:, 0:kb, :], in_=yj).then_inc(
                dsem[i % BT], 16
            )

        @block.sync
        def _(sync):
            for i in range(n):
                dma_in(sync, i)
            sync.wait_ge(s_r, SPLIT)
            sync.dma_start(
                out=out[:, 0:cols_split], in_=obuf.ap()[:, 0:cols_split]
            ).then_inc(s_out, 16)
            sync.wait_ge(s_r, n)
            sync.dma_start(
                out=out[:, cols_split:], in_=obuf.ap()[:, cols_split:]
            ).then_inc(s_out, 16)
            sync.wait_ge(s_out, 32)
            sync.drain(semaphore_range=range(sem_nums[0], sem_nums[-1] + 1))
            sync.sem_clear(range(sem_nums[0], sem_nums[-1] + 1))

        @block.scalar
        def _(scalar):
            # Dummy 1-elem exp: forces the ACT_TABLE_LOAD to run immediately
            # at body start, overlapping the first input DMA.
            scalar.activation(dum.ap()[:, 0:1], dum.ap()[:, 0:1], Exp)
            for i, j in enumerate(jobs):
                kb, f = j["kb"], j["f"]
                na = kb - f  # ACT rows
                if na == 0:
                    continue
                scalar.wait_ge(dsem[i % BT], 16 * (i // BT + 1))
                if i >= BE:
                    # E slot reuse: h1 of job i-BE must have consumed E.
                    k = i - BE
                    if h1_gps[k]:
                        scalar.wait_ge(s_g, g_after_h1[k])
                    else:
                        scalar.wait_ge(s_v, v_after_h1[k])
                scalar.activation(
                    E_s[i % BE].ap()[:, 0:na, :], T_s[i % BT].ap()[:, 0:na, :], Exp
                ).then_inc(s_exp, 1)

        @block.gpsimd
        def _(gp):
            for i, j in enumerate(jobs):
                if not h1_gps[i]:
                    continue
                kb = j["kb"]
                E = E_s[i % BE].ap()
                H = H_s[i % BH].ap()
                if has_exp[i]:
                    gp.wait_ge(s_exp, nexp[i])
                if has_ts[i]:
                    gp.wait_ge(s_ts, nts[i])
                if i >= BH:
                    # H slot reuse: h2 of job i-BH consumed it.
                    k = i - BH
                    if not h2_gps[k]:
                        gp.wait_ge(s_v, v_after_h2[k])
                gp.tensor_add(
                    H[:, 0:kb, :], E[:, 0:kb, 0 : C // 2], E[:, 0:kb, C // 2 : C]
                ).then_inc(s_g, 1)
                if h2_gps[i]:
                    if i >= BH:
                        # H2 slot reuse: h3 (always DVE) of job i-BH.
                        gp.wait_ge(s_v, v_after_h3[i - BH])
                    H2 = H2_s[i % BH].ap()
                    gp.tensor_add(
                        H2[:, 0:kb, :],
                        H[:, 0:kb, 0 : C // 4],
                        H[:, 0:kb, C // 4 : C // 2],
                    ).then_inc(s_g, 1)

        @block.vector
        def _(vec):
            for i, j in enumerate(jobs):
                kb, f = j["kb"], j["f"]
                E = E_s[i % BE].ap()
                H = H_s[i % BH].ap()
                H2 = H2_s[i % BH].ap()
                H3 = H3_s[i % BH].ap()
                if f > 0:
                    # Schraudolph fast-exp: one dual-op tensor_scalar, fp16 in,
                    # int16 out (the convert IS the exp). E slot already safe
                    # if our own h1(i-BE) ran on DVE (in-order); else wait GpS.
                    vec.wait_ge(dsem[i % BT], 16 * (i // BT + 1))
                    if i >= BE and h1_gps[i - BE]:
                        vec.wait_ge(s_g, g_after_h1[i - BE])
                    Ei = E.bitcast(i16)
                    vec.tensor_scalar(
                        Ei[:, kb - f : kb, :],
                        T_s[i % BT].ap()[:, kb - f : kb, :],
                        FE_A,
                        FE_B,
                        Mult,
                        Add,
                    ).then_inc(s_ts, 1)
                # chain
                if not h1_gps[i]:
                    if has_exp[i]:
                        vec.wait_ge(s_exp, nexp[i])
                    if i >= BH and h2_gps[i - BH]:
                        vec.wait_ge(s_g, g_after_h2[i - BH])
                    vec.tensor_add(
                        H[:, 0:kb, :], E[:, 0:kb, 0 : C // 2], E[:, 0:kb, C // 2 : C]
                    ).then_inc(s_v, 1)
                if not h2_gps[i]:
                    if h1_gps[i]:
                        vec.wait_ge(s_g, g_after_h1[i])
                    vec.tensor_add(
                        H2[:, 0:kb, :],
                        H[:, 0:kb, 0 : C // 4],
                        H[:, 0:kb, C // 4 : C // 2],
                    ).then_inc(s_v, 1)
                # h3 always DVE
                if h2_gps[i]:
                    vec.wait_ge(s_g, g_after_h2[i])
                vec.tensor_add(
                    H3[:, 0:kb, :],
                    H2[:, 0:kb, 0 : C // 8],
                    H2[:, 0:kb, C // 8 : C // 4],
                ).then_inc(s_v, 1)
                vec.tensor_reduce(
                    obuf.ap()[:, cols[i] : cols[i] + kb],
                    H3[:, 0:kb, :],
                    axis=X,
                    op=Add,
                ).then_inc(s_r, 1)

    nc.finalize()
    return nc


def _get_nc():
    if "nc" not in _CACHE:
        _CACHE["nc"] = _build_nc()
    return _CACHE["nc"]


def _make_in_maps(y_pred: np.ndarray):
    y16 = np.asarray(y_pred).astype(np.float16)
    return [
        {"y_pred": np.ascontiguousarray(y16[c * M : (c + 1) * M])}
        for c in range(N_CORES)
    ]


def _run(in_maps, trace=False, **kwargs):
    from concourse.bass_utils import run_bass_kernel_spmd

    nc = _get_nc()
    return run_bass_kernel_spmd(
        nc, in_maps, list(range(N_CORES)), trace=trace, **kwargs
    )


def _combine(results, y_pred: np.ndarray, y_true: np.ndarray) -> np.ndarray:
    yp = np.asarray(y_pred)
    yt = np.asarray(y_true).reshape(-1).astype(np.int64)

    # Per-row sumexp from the device: out[p, col], col layout per JOBS.
    rowmap = np.empty((P, M // P), dtype=np.int64)
    col = 0
    for j in JOBS:
        kb = j["kb"]
        rowmap[:, col : col + kb] = (
            j["base"] + np.arange(P)[:, None] * kb + np.arange(kb)[None, :]
        )
        col += kb
    lse = np.empty(N, dtype=np.float64)
    for c in range(N_CORES):
        o = np.log(results[c]["out"].astype(np.float64))  # [P, M // P]
        lse[c * M + rowmap.reshape(-1)] = o.reshape(-1)

    picked = (
        np.take_along_axis(yp, yt[:, None], axis=1).reshape(-1).astype(np.float64)
    )
    ce = -(picked.sum() - lse.sum()) / N

    p1 = np.exp(yp[:, 0].astype(np.float64) - lse)
    lp = np.log(p1 + EPS)
    lq = np.log((1.0 + EPS) - p1)
    nj = np.bincount(yt, minlength=C).astype(np.float64)
    s = BETA * (1.0 - nj / (N - nj[0]))
    v = np.where(yt == 0, ALPHA * lp, s[yt] * lq)
    loss = ce - v.sum() / N
    return np.asarray(loss, dtype=np.float32)


def kernel(y_pred: np.ndarray, y_true: np.ndarray) -> np.ndarray:
    in_maps = _make_in_maps(y_pred)
    res = _run(in_maps, trace=False)
    return _combine(res.results, y_pred, y_true)


# revision 6
# speedup vs baseline: 1.2740x; 1.2740x over previous
"""Trainium2 Bass kernel for nn_CustomLoss_19061064859882.

loss = CE(y_pred, y_true) - penalty/N, where the penalty uses
p1 = softmax(y_pred)[:, 0] and per-class weights from the label histogram.

Device computes per-row partial sumexp over the 128 classes (data-parallel
over the 8 cores, fp16 wire): each row's 128 exps are reduced on-device to
16 partial sums (h1+h2+h3 pairwise-add chain); the host finishes the last 4
adds and the log in float64, along with the other O(N) bookkeeping (picked
logits, label bincount, final scalar).

Two-engine design (GpSimd is deliberately idle: the POOL slot shares the
SBUF read port with the DVE, so concurrent GpSimd tensor ops degrade DVE
2-port instructions ~3.4x, measured -- strictly counterproductive):
  - ACT (scalar) runs native EXP on most rows (1 elem/cycle/lane).
  - DVE (vector) runs a Schraudolph fast-exp on the rest in ONE 4x-mode
    tensor_scalar op: i16 = int16(x * 1477.32 + 15301.3); the int16 bit
    pattern reinterpreted as fp16 approximates e^x (rel err ~2.4%, zero-mean
    after calibrating the additive constant; final loss error ~2e-4, far
    inside the 2e-2 tolerance). DVE also runs the halving chain, writing h3
    results (16 fp16 partials per row) straight into the output buffer.
Jobs are kb=48 to amortize per-instruction overheads, the last jobs are
small to shorten the post-EXP chain tail, and the out-DMA is split in three
so all but the last ~0.15 MiB overlaps compute.
"""

import sys

import numpy as np

if "/opt/trn_rl_repo" not in sys.path:
    sys.path.insert(0, "/opt/trn_rl_repo")

N_CORES = 8
N = 262144
C = 128  # classes
M = N // N_CORES  # rows per core (32768)
P = 128  # SBUF partitions
K_OUT = 16  # partial sums kept per row (device reduces 128 -> 16)
ALPHA = 0.5
BETA = 0.5
EPS = 1e-9

# Schraudolph fast-exp constants (fp16 bit trick), calibrated so the MEAN of
# log(row_sum_approx) - lse is ~0 on N(0,1) logits.
FE_A = 1477.3197218702985  # 1024 * log2(e)
FE_B = 15301.33  # 1024 * (15 - delta), delta ~= 0.0573

# Jobs: kb = rows per partition, f = rows via DVE fast-exp (rest on ACT).
JOBS = [
    dict(kb=8, f=0),
    dict(kb=16, f=0),
    dict(kb=48, f=13),
    dict(kb=48, f=13),
    dict(kb=48, f=13),
    dict(kb=48, f=13),
    dict(kb=32, f=7),
    dict(kb=8, f=8),  # tail job, all-DVE (no EXP)
]
_base = 0
for _j in JOBS:
    _j["base"] = _base
    _base += P * _j["kb"]
assert _base == M

BT = 4  # T (input) buffer slots
BE = 4  # E (exp) buffer slots
BH = 3  # H/H2 chain buffer slots

# Vector emission order (op level): chains in job order, ts ops as soon as
# their DMA can have landed, the tiny all-DVE job 7 hoisted before job 6's
# chain so the final tail is one short kb=32 chain.
VEC_PROG = [
    ("c", 0),
    ("c", 1),
    ("ts", 2),
    ("c", 2),
    ("ts", 3),
    ("c", 3),
    ("ts", 4),
    ("c", 4),
    ("ts", 5),
    ("ts", 6),
    ("ts", 7),
    ("c", 5),
    ("c", 7),
    ("c", 6),
]

_CACHE: dict = {}


def _build_nc():
    import contextlib

    import concourse.bacc as bacc
    import concourse.mybir as mybir

    f16 = mybir.dt.float16
    i16 = mybir.dt.int16
    Exp = mybir.ActivationFunctionType.Exp
    Add = mybir.AluOpType.add
    Mult = mybir.AluOpType.mult

    nc = bacc.Bacc(
        "TRN2", target_bir_lowering=False, debug=False, num_devices=N_CORES
    )
    y = nc.dram_tensor("y_pred", [M, C], f16, kind="ExternalInput").ap()
    out = nc.dram_tensor(
        "out", [P, (M // P) * K_OUT], f16, kind="ExternalOutput"
    ).ap()

    KBMAX = 48
    T_s = [nc.alloc_sbuf_tensor(f"Tb{i}", [P, KBMAX, C], f16) for i in range(BT)]
    E_s = [nc.alloc_sbuf_tensor(f"Eb{i}", [P, KBMAX, C], f16) for i in range(BE)]
    H_s = [nc.alloc_sbuf_tensor(f"Hb{i}", [P, KBMAX, C // 2], f16) for i in range(BH)]
    H2_s = [nc.alloc_sbuf_tensor(f"H2b{i}", [P, KBMAX, C // 4], f16) for i in range(BH)]
    obuf = nc.alloc_sbuf_tensor("obuf", [P, M // P, K_OUT], f16)
    dum = nc.alloc_sbuf_tensor("dum", [P, 1], f16)

    jobs = list(JOBS)
    n = len(jobs)
    has_exp = [j["kb"] - j["f"] > 0 for j in jobs]
    has_ts = [j["f"] > 0 for j in jobs]
    nexp = np.cumsum(has_exp).tolist()

    # Emission-order cumulative counts. s_ts: +1 per ts. s_v: +1 per h1
    # (E-slot release marker). s_r: +1 per h3 (obuf written).
    ts_pos: dict[int, int] = {}
    h1_pos: dict[int, int] = {}
    r_pos: dict[int, int] = {}
    cts = cv = cr = 0
    for kind, i in VEC_PROG:
        if kind == "ts":
            assert has_ts[i]
            cts += 1
            ts_pos[i] = cts
        else:
            cv += 1
            h1_pos[i] = cv
            cr += 1
            r_pos[i] = cr
    assert cts == sum(has_ts) and cr == n

    cols = []
    col = 0
    for j in jobs:
        cols.append(col)
        col += j["kb"]
    assert col == M // P

    # Out-DMA split points: after jobs {0..3}, {4,5}, rest (emission order of
    # h3 is 0,1,2,3,4,5,7,6 so s_r>=4 / >=6 / >=8 gate these column ranges).
    assert [i for k, i in VEC_PROG if k == "c"] == [0, 1, 2, 3, 4, 5, 7, 6]
    SPLITS = [(4, 0, cols[4]), (6, cols[4], cols[6]), (8, cols[6], M // P)]

    with contextlib.ExitStack() as stack:
        block = stack.enter_context(nc.Block())
        dsem = [stack.enter_context(nc.semaphore(f"s_dma{i}")) for i in range(BT)]
        s_exp = stack.enter_context(nc.semaphore("s_exp"))
        s_ts = stack.enter_context(nc.semaphore("s_ts"))
        s_v = stack.enter_context(nc.semaphore("s_v"))
        s_r = stack.enter_context(nc.semaphore("s_r"))
        s_out = stack.enter_context(nc.semaphore("s_out"))
        all_sems = dsem + [s_exp, s_ts, s_v, s_r, s_out]
        sem_nums = sorted(s.num for s in all_sems)

        @block.sync
        def _(sync):
            for i, j in enumerate(jobs):
                base, kb = j["base"], j["kb"]
                if i >= BT:
                    k = i - BT
                    if has_exp[k]:
                        sync.wait_ge(s_exp, nexp[k])
                    if has_ts[k]:
                        sync.wait_ge(s_ts, ts_pos[k])
                yj = y[base : base + P * kb].rearrange("(p k) c -> p k c", p=P)
                sync.dma_start(out=T_s[i % BT].ap()[:, 0:kb, :], in_=yj).then_inc(
                    dsem[i % BT], 16
                )
            for si, (rcnt, c0, c1) in enumerate(SPLITS):
                sync.wait_ge(s_r, rcnt)
                sync.dma_start(
                    out=out[:, c0 * K_OUT : c1 * K_OUT],
                    in_=obuf.ap()[:, c0:c1, :],
                ).then_inc(s_out, 16)
            sync.wait_ge(s_out, 16 * len(SPLITS))
            sync.drain(semaphore_range=range(sem_nums[0], sem_nums[-1] + 1))
            sync.sem_clear(range(sem_nums[0], sem_nums[-1] + 1))

        @block.scalar
        def _(scalar):
            # Dummy 1-elem exp: forces ACT_TABLE_LOAD at body start,
            # overlapping the first input DMAs.
            scalar.activation(dum.ap()[:, 0:1], dum.ap()[:, 0:1], Exp)
            for i, j in enumerate(jobs):
                kb, f = j["kb"], j["f"]
                na = kb - f
                if na == 0:
                    continue
                scalar.wait_ge(dsem[i % BT], 16 * (i // BT + 1))
                if i >= BE:
                    scalar.wait_ge(s_v, h1_pos[i - BE])
                scalar.activation(
                    E_s[i % BE].ap()[:, 0:na, :], T_s[i % BT].ap()[:, 0:na, :], Exp
                ).then_inc(s_exp, 1)

        @block.vector
        def _(vec):
            for kind, i in VEC_PROG:
                j = jobs[i]
                kb, f = j["kb"], j["f"]
                E = E_s[i % BE].ap()
                H = H_s[i % BH].ap()
                H2 = H2_s[i % BH].ap()
                if kind == "ts":
                    vec.wait_ge(dsem[i % BT], 16 * (i // BT + 1))
                    if i >= BE:
                        assert h1_pos.get(i - BE) is not None  # emitted earlier
                    Ei = E.bitcast(i16)
                    vec.tensor_scalar(
                        Ei[:, kb - f : kb, :],
                        T_s[i % BT].ap()[:, kb - f : kb, :],
                        FE_A,
                        FE_B,
                        Mult,
                        Add,
                    ).then_inc(s_ts, 1)
                    continue
                if has_exp[i]:
                    vec.wait_ge(s_exp, nexp[i])
                vec.tensor_add(
                    H[:, 0:kb, :], E[:, 0:kb, 0 : C // 2], E[:, 0:kb, C // 2 : C]
                ).then_inc(s_v, 1)
                vec.tensor_add(
                    H2[:, 0:kb, :],
                    H[:, 0:kb, 0 : C // 4],
                    H[:, 0:kb, C // 4 : C // 2],
                )
                vec.tensor_add(
                    obuf.ap()[:, cols[i] : cols[i] + kb, :],
                    H2[:, 0:kb, 0 : C // 8],
                    H2[:, 0:kb, C // 8 : C // 4],
                ).then_inc(s_r, 1)

    nc.finalize()
    return nc


def _get_nc():
    if "nc" not in _CACHE:
        _CACHE["nc"] = _build_nc()
    return _CACHE["nc"]


def _make_in_maps(y_pred: np.ndarray):
    y16 = np.asarray(y_pred).astype(np.float16)
    return [
        {"y_pred": np.ascontiguousarray(y16[c * M : (c + 1) * M])}
        for c in range(N_CORES)
    ]


def _run(in_maps, trace=False, **kwargs):
    from concourse.bass_utils import run_bass_kernel_spmd

    nc = _get_nc()
    return run_bass_kernel_spmd(
        nc, in_maps, list(range(N_CORES)), trace=trace, **kwargs
    )


def _combine(results, y_pred: np.ndarray, y_true: np.ndarray) -> np.ndarray:
    yp = np.asarray(y_pred)
    yt = np.asarray(y_true).reshape(-1).astype(np.int64)

    rowmap = np.empty((P, M // P), dtype=np.int64)
    col = 0
    for j in JOBS:
        kb = j["kb"]
        rowmap[:, col : col + kb] = (
            j["base"] + np.arange(P)[:, None] * kb + np.arange(kb)[None, :]
        )
        col += kb
    lse = np.empty(N, dtype=np.float64)
    for c in range(N_CORES):
        o = results[c]["out"].astype(np.float64).reshape(P, M // P, K_OUT)
        lse[c * M + rowmap.reshape(-1)] = np.log(o.sum(axis=2)).reshape(-1)

    picked = (
        np.take_along_axis(yp, yt[:, None], axis=1).reshape(-1).astype(np.float64)
    )
    ce = -(picked.sum() - lse.sum()) / N

    p1 = np.exp(yp[:, 0].astype(np.float64) - lse)
    lp = np.log(p1 + EPS)
    lq = np.log((1.0 + EPS) - p1)
    nj = np.bincount(yt, minlength=C).astype(np.float64)
    s = BETA * (1.0 - nj / (N - nj[0]))
    v = np.where(yt == 0, ALPHA * lp, s[yt] * lq)
    loss = ce - v.sum() / N
    return np.asarray(loss, dtype=np.float32)


def kernel(y_pred: np.ndarray, y_true: np.ndarray) -> np.ndarray:
    in_maps = _make_in_maps(y_pred)
    res = _run(in_maps, trace=False)
    return _combine(res.results, y_pred, y_true)


# revision 7
# speedup vs baseline: 1.3842x; 1.0865x over previous
"""Trainium2 Bass kernel for nn_CustomLoss_19061064859882.

loss = CE(y_pred, y_true) - penalty/N, where the penalty uses
p1 = softmax(y_pred)[:, 0] and per-class weights from the label histogram.

Device computes per-row partial sumexp over the 128 classes (data-parallel
over the 8 cores): each row's 128 exps are reduced on-device to 16 partial
sums (h1+h2+h3 pairwise-add chain); the host finishes the last 4 adds and
the log in float64, plus the other O(N) bookkeeping (picked logits, label
bincount, final scalar).

Wire format: rows destined for the ACT engine travel as fp8 e4m3 (exp of
an e4m3-rounded N(0,1) logit costs ~1e-3 in log-sum accuracy -- measured
std 0.008, bias -8e-5 -- far inside the 2e-2 tolerance), halving input DMA
for those rows. Rows destined for the DVE fast-exp stay fp16 (the 4x-mode
tensor_scalar needs 16-bit operands).

Two-engine compute (GpSimd deliberately idle: the POOL slot shares the
SBUF read port with the DVE, so concurrent GpSimd tensor ops degrade DVE
2-port instructions ~3.4x, measured):
  - ACT (scalar): native EXP, fp8 in / fp16 out, 1 elem/cycle/lane.
  - DVE (vector): Schraudolph fast-exp in ONE 4x-mode tensor_scalar op
    (i16 = int16(x*1477.32 + 15301.3), bit pattern read back as fp16), plus
    the whole halving chain, h3 writing 16 fp16 partials per row straight
    into the output buffer.
Jobs are kb=48 to amortize per-instruction overheads, the last jobs are
small to shorten the post-EXP chain tail, and the out-DMA is split in three
so all but the last ~0.16 MiB overlaps compute.
"""

import sys

import numpy as np

if "/opt/trn_rl_repo" not in sys.path:
    sys.path.insert(0, "/opt/trn_rl_repo")

N_CORES = 8
N = 262144
C = 128  # classes
M = N // N_CORES  # rows per core (32768)
P = 128  # SBUF partitions
K_OUT = 16  # partial sums kept per row (device reduces 128 -> 16)
ALPHA = 0.5
BETA = 0.5
EPS = 1e-9

# Schraudolph fast-exp constants (fp16 bit trick), calibrated so the MEAN of
# log(row_sum_approx) - lse is ~0 on N(0,1) logits.
FE_A = 1477.3197218702985  # 1024 * log2(e)
FE_B = 15301.33  # 1024 * (15 - delta), delta ~= 0.0573

# Jobs: kb = rows per partition, f = rows via DVE fast-exp (rest on ACT).
JOBS = [
    dict(kb=8, f=0),
    dict(kb=16, f=0),
    dict(kb=48, f=15),
    dict(kb=48, f=15),
    dict(kb=48, f=15),
    dict(kb=48, f=15),
    dict(kb=32, f=9),
    dict(kb=8, f=8),  # tail job, all-DVE (no EXP)
]
_base = _b8 = _b16 = 0
for _j in JOBS:
    _j["base"] = _base
    _j["base8"] = _b8
    _j["base16"] = _b16
    _base += P * _j["kb"]
    _b8 += P * (_j["kb"] - _j["f"])
    _b16 += P * _j["f"]
assert _base == M
M8, M16 = _b8, _b16

BT = 4  # T8/T16 (input) buffer slots
BE = 4  # E (exp) buffer slots
BH = 3  # H/H2 chain buffer slots
FMAX = 16

# Vector emission order (op level): chains in job order, ts ops as soon as
# their DMA can have landed, the tiny all-DVE job 7 hoisted before job 6's
# chain so the final tail is one short kb=32 chain.
VEC_PROG = [
    ("c", 0),
    ("c", 1),
    ("ts", 2),
    ("c", 2),
    ("ts", 3),
    ("c", 3),
    ("ts", 4),
    ("c", 4),
    ("ts", 5),
    ("ts", 6),
    ("ts", 7),
    ("c", 5),
    ("c", 7),
    ("c", 6),
]

_CACHE: dict = {}


def _build_nc():
    import contextlib

    import concourse.bacc as bacc
    import concourse.mybir as mybir

    f16 = mybir.dt.float16
    f8 = mybir.dt.float8e4
    i16 = mybir.dt.int16
    Exp = mybir.ActivationFunctionType.Exp
    Add = mybir.AluOpType.add
    Mult = mybir.AluOpType.mult

    nc = bacc.Bacc(
        "TRN2", target_bir_lowering=False, debug=False, num_devices=N_CORES
    )
    y8 = nc.dram_tensor("y8", [M8, C], f8, kind="ExternalInput").ap()
    y16 = nc.dram_tensor("y16", [M16, C], f16, kind="ExternalInput").ap()
    out = nc.dram_tensor(
        "out", [P, (M // P) * K_OUT], f16, kind="ExternalOutput"
    ).ap()

    KBMAX = 48
    T8_s = [nc.alloc_sbuf_tensor(f"T8b{i}", [P, KBMAX, C], f8) for i in range(BT)]
    T16_s = [
        nc.alloc_sbuf_tensor(f"T16b{i}", [P, FMAX, C], f16) for i in range(BT)
    ]
    E_s = [nc.alloc_sbuf_tensor(f"Eb{i}", [P, KBMAX, C], f16) for i in range(BE)]
    H_s = [nc.alloc_sbuf_tensor(f"Hb{i}", [P, KBMAX, C // 2], f16) for i in range(BH)]
    H2_s = [nc.alloc_sbuf_tensor(f"H2b{i}", [P, KBMAX, C // 4], f16) for i in range(BH)]
    obuf = nc.alloc_sbuf_tensor("obuf", [P, M // P, K_OUT], f16)
    dum = nc.alloc_sbuf_tensor("dum", [P, 1], f16)

    jobs = list(JOBS)
    n = len(jobs)
    has_exp = [j["kb"] - j["f"] > 0 for j in jobs]
    has_ts = [j["f"] > 0 for j in jobs]
    nexp = np.cumsum(has_exp).tolist()

    # Emission-order cumulative counts. s_ts: +1 per ts. s_v: +1 per h1
    # (E-slot release marker). s_r: +1 per h3 (obuf written).
    ts_pos: dict[int, int] = {}
    h1_pos: dict[int, int] = {}
    cts = cv = cr = 0
    for kind, i in VEC_PROG:
        if kind == "ts":
            assert has_ts[i]
            cts += 1
            ts_pos[i] = cts
        else:
            cv += 1
            h1_pos[i] = cv
            cr += 1
    assert cts == sum(has_ts) and cr == n

    cols = []
    col = 0
    for j in jobs:
        cols.append(col)
        col += j["kb"]
    assert col == M // P

    # Out-DMA split points (emission order of h3 is 0,1,2,3,4,5,7,6).
    assert [i for k, i in VEC_PROG if k == "c"] == [0, 1, 2, 3, 4, 5, 7, 6]
    SPLITS = [(4, 0, cols[4]), (6, cols[4], cols[6]), (8, cols[6], M // P)]

    with contextlib.ExitStack() as stack:
        block = stack.enter_context(nc.Block())
        dsem8 = [stack.enter_context(nc.semaphore(f"s_d8{i}")) for i in range(BT)]
        dsem16 = [stack.enter_context(nc.semaphore(f"s_d16{i}")) for i in range(BT)]
        s_exp = stack.enter_context(nc.semaphore("s_exp"))
        s_ts = stack.enter_context(nc.semaphore("s_ts"))
        s_v = stack.enter_context(nc.semaphore("s_v"))
        s_r = stack.enter_context(nc.semaphore("s_r"))
        s_out = stack.enter_context(nc.semaphore("s_out"))
        all_sems = dsem8 + dsem16 + [s_exp, s_ts, s_v, s_r, s_out]
        sem_nums = sorted(s.num for s in all_sems)

        # Per-slot DMA pass counters (for cumulative dsem targets).
        pass8 = [0] * BT
        pass16 = [0] * BT
        d8_target = [0] * n
        d16_target = [0] * n
        for i, j in enumerate(jobs):
            if has_exp[i]:
                pass8[i % BT] += 1
                d8_target[i] = 16 * pass8[i % BT]
            if has_ts[i]:
                pass16[i % BT] += 1
                d16_target[i] = 16 * pass16[i % BT]

        @block.sync
        def _(sync):
            for i, j in enumerate(jobs):
                kb, f = j["kb"], j["f"]
                na = kb - f
                if i >= BT:
                    k = i - BT
                    if has_exp[k]:
                        sync.wait_ge(s_exp, nexp[k])
                    if has_ts[k]:
                        sync.wait_ge(s_ts, ts_pos[k])
                if na > 0:
                    ya = y8[j["base8"] : j["base8"] + P * na].rearrange(
                        "(p k) c -> p k c", p=P
                    )
                    sync.dma_start(
                        out=T8_s[i % BT].ap()[:, 0:na, :], in_=ya
                    ).then_inc(dsem8[i % BT], 16)
                if f > 0:
                    yb = y16[j["base16"] : j["base16"] + P * f].rearrange(
                        "(p k) c -> p k c", p=P
                    )
                    sync.dma_start(
                        out=T16_s[i % BT].ap()[:, 0:f, :], in_=yb
                    ).then_inc(dsem16[i % BT], 16)
            for rcnt, c0, c1 in SPLITS:
                sync.wait_ge(s_r, rcnt)
                sync.dma_start(
                    out=out[:, c0 * K_OUT : c1 * K_OUT],
                    in_=obuf.ap()[:, c0:c1, :],
                ).then_inc(s_out, 16)
            sync.wait_ge(s_out, 16 * len(SPLITS))
            sync.drain(semaphore_range=range(sem_nums[0], sem_nums[-1] + 1))
            sync.sem_clear(range(sem_nums[0], sem_nums[-1] + 1))

        @block.scalar
        def _(scalar):
            # Dummy 1-elem exp: forces ACT_TABLE_LOAD at body start,
            # overlapping the first input DMAs.
            scalar.activation(dum.ap()[:, 0:1], dum.ap()[:, 0:1], Exp)
            for i, j in enumerate(jobs):
                kb, f = j["kb"], j["f"]
                na = kb - f
                if na == 0:
                    continue
                scalar.wait_ge(dsem8[i % BT], d8_target[i])
                if i >= BE:
                    scalar.wait_ge(s_v, h1_pos[i - BE])
                scalar.activation(
                    E_s[i % BE].ap()[:, 0:na, :], T8_s[i % BT].ap()[:, 0:na, :], Exp
                ).then_inc(s_exp, 1)

        @block.vector
        def _(vec):
            for kind, i in VEC_PROG:
                j = jobs[i]
                kb, f = j["kb"], j["f"]
                E = E_s[i % BE].ap()
                H = H_s[i % BH].ap()
                H2 = H2_s[i % BH].ap()
                if kind == "ts":
                    vec.wait_ge(dsem16[i % BT], d16_target[i])
                    if i >= BE:
                        assert h1_pos.get(i - BE) is not None  # emitted earlier
                    Ei = E.bitcast(i16)
                    vec.tensor_scalar(
                        Ei[:, kb - f : kb, :],
                        T16_s[i % BT].ap()[:, 0:f, :],
                        FE_A,
                        FE_B,
                        Mult,
                        Add,
                    ).then_inc(s_ts, 1)
                    continue
                if has_exp[i]:
                    vec.wait_ge(s_exp, nexp[i])
                vec.tensor_add(
                    H[:, 0:kb, :], E[:, 0:kb, 0 : C // 2], E[:, 0:kb, C // 2 : C]
                ).then_inc(s_v, 1)
                vec.tensor_add(
                    H2[:, 0:kb, :],
                    H[:, 0:kb, 0 : C // 4],
                    H[:, 0:kb, C // 4 : C // 2],
                )
                vec.tensor_add(
                    obuf.ap()[:, cols[i] : cols[i] + kb, :],
                    H2[:, 0:kb, 0 : C // 8],
                    H2[:, 0:kb, C // 8 : C // 4],
                ).then_inc(s_r, 1)

    nc.finalize()
    return nc


def _get_nc():
    if "nc" not in _CACHE:
        _CACHE["nc"] = _build_nc()
    return _CACHE["nc"]


def _make_in_maps(y_pred: np.ndarray):
    import ml_dtypes

    yp = np.asarray(y_pred)
    maps = []
    for c in range(N_CORES):
        yc = yp[c * M : (c + 1) * M]
        parts8, parts16 = [], []
        for j in JOBS:
            kb, f = j["kb"], j["f"]
            na = kb - f
            blk = yc[j["base"] : j["base"] + P * kb].reshape(P, kb, C)
            if na > 0:
                parts8.append(blk[:, 0:na, :].reshape(-1, C))
            if f > 0:
                parts16.append(blk[:, na:kb, :].reshape(-1, C))
        y8 = np.concatenate(parts8).astype(ml_dtypes.float8_e4m3)
        y16 = np.concatenate(parts16).astype(np.float16)
        maps.append(
            {"y8": np.ascontiguousarray(y8), "y16": np.ascontiguousarray(y16)}
        )
    return maps


def _run(in_maps, trace=False, **kwargs):
    from concourse.bass_utils import run_bass_kernel_spmd

    nc = _get_nc()
    return run_bass_kernel_spmd(
        nc, in_maps, list(range(N_CORES)), trace=trace, **kwargs
    )


def _combine(results, y_pred: np.ndarray, y_true: np.ndarray) -> np.ndarray:
    yp = np.asarray(y_pred)
    yt = np.asarray(y_true).reshape(-1).astype(np.int64)

    rowmap = np.empty((P, M // P), dtype=np.int64)
    col = 0
    for j in JOBS:
        kb = j["kb"]
        rowmap[:, col : col + kb] = (
            j["base"] + np.arange(P)[:, None] * kb + np.arange(kb)[None, :]
        )
        col += kb
    lse = np.empty(N, dtype=np.float64)
    for c in range(N_CORES):
        o = results[c]["out"].astype(np.float64).reshape(P, M // P, K_OUT)
        lse[c * M + rowmap.reshape(-1)] = np.log(o.sum(axis=2)).reshape(-1)

    picked = (
        np.take_along_axis(yp, yt[:, None], axis=1).reshape(-1).astype(np.float64)
    )
    ce = -(picked.sum() - lse.sum()) / N

    p1 = np.exp(yp[:, 0].astype(np.float64) - lse)
    lp = np.log(p1 + EPS)
    lq = np.log((1.0 + EPS) - p1)
    nj = np.bincount(yt, minlength=C).astype(np.float64)
    s = BETA * (1.0 - nj / (N - nj[0]))
    v = np.where(yt == 0, ALPHA * lp, s[yt] * lq)
    loss = ce - v.sum() / N
    return np.asarray(loss, dtype=np.float32)


def kernel(y_pred: np.ndarray, y_true: np.ndarray) -> np.ndarray:
    in_maps = _make_in_maps(y_pred)
    res = _run(in_maps, trace=False)
    return _combine(res.results, y_pred, y_true)


# revision 8
# speedup vs baseline: 1.5021x; 1.0852x over previous
"""Trainium2 Bass kernel for nn_CustomLoss_19061064859882.

loss = CE(y_pred, y_true) - penalty/N, where the penalty uses
p1 = softmax(y_pred)[:, 0] and per-class weights from the label histogram.

Device computes per-row partial sumexp over the 128 classes (data-parallel
over the 8 cores): each row's 128 exps are reduced on-device to 32 partial
sums (h1+h2 pairwise-add chain); the host finishes the remaining adds and
the log in float64, plus the other O(N) bookkeeping (picked logits, label
bincount, final scalar).

Wire format: rows destined for the ACT engine travel as fp8 e4m3 (costs
~1e-3 in log-sum accuracy -- measured std 0.008, bias -8e-5 -- far inside
the 2e-2 tolerance), halving input DMA for those rows. Rows destined for
the DVE fast-exp stay fp16 (the 4x-mode tensor_scalar needs 16-bit).

Two-engine compute (GpSimd deliberately idle: the POOL slot shares the
SBUF read port with the DVE, so concurrent GpSimd tensor ops degrade DVE
2-port instructions ~3.4x, measured):
  - ACT (scalar): native EXP, fp8 in / fp16 out, 1 elem/cycle/lane.
  - DVE (vector): Schraudolph fast-exp in ONE 4x-mode tensor_scalar op
    (i16 = int16(x*1477.32 + 15301.3), bit pattern read back as fp16), plus
    the h1+h2 halving chain, h2 writing 32 fp16 partials per row straight
    into the output buffer.
The first job is kb=32 (one sizable DMA amortizes the cold-ring latency),
the last jobs are small to shorten the post-EXP chain tail, and the
out-DMA is split in three so all but the last ~0.25 MiB overlaps compute.
"""

import sys

import numpy as np

if "/opt/trn_rl_repo" not in sys.path:
    sys.path.insert(0, "/opt/trn_rl_repo")

N_CORES = 8
N = 262144
C = 128  # classes
M = N // N_CORES  # rows per core (32768)
P = 128  # SBUF partitions
K_OUT = 32  # partial sums kept per row (device reduces 128 -> 32)
ALPHA = 0.5
BETA = 0.5
EPS = 1e-9

# Schraudolph fast-exp constants (fp16 bit trick), calibrated so the MEAN of
# log(row_sum_approx) - lse is ~0 on N(0,1) logits.
FE_A = 1477.3197218702985  # 1024 * log2(e)
FE_B = 15301.33  # 1024 * (15 - delta), delta ~= 0.0573

# Jobs: kb = rows per partition, f = rows via DVE fast-exp (rest on ACT).
JOBS = [
    dict(kb=32, f=8),
    dict(kb=48, f=18),
    dict(kb=48, f=18),
    dict(kb=48, f=18),
    dict(kb=48, f=18),
    dict(kb=24, f=6),
    dict(kb=8, f=8),  # tail job, all-DVE (no EXP)
]
_base = _b8 = _b16 = 0
for _j in JOBS:
    _j["base"] = _base
    _j["base8"] = _b8
    _j["base16"] = _b16
    _base += P * _j["kb"]
    _b8 += P * (_j["kb"] - _j["f"])
    _b16 += P * _j["f"]
assert _base == M
M8, M16 = _b8, _b16

BT = 4  # T8/T16 (input) buffer slots
BE = 4  # E (exp) buffer slots
BH = 3  # H chain buffer slots
FMAX = 20

# Vector emission order (op level): per-job [ts, chain], with the tiny
# all-DVE job 6 hoisted before job 5's chain so the final tail is short.
VEC_PROG = [
    ("ts", 0),
    ("c", 0),
    ("ts", 1),
    ("c", 1),
    ("ts", 2),
    ("c", 2),
    ("ts", 3),
    ("c", 3),
    ("ts", 4),
    ("ts", 5),
    ("ts", 6),
    ("c", 4),
    ("c", 6),
    ("c", 5),
]

_CACHE: dict = {}


def _build_nc():
    import contextlib

    import concourse.bacc as bacc
    import concourse.mybir as mybir

    f16 = mybir.dt.float16
    f8 = mybir.dt.float8e4
    i16 = mybir.dt.int16
    Exp = mybir.ActivationFunctionType.Exp
    Add = mybir.AluOpType.add
    Mult = mybir.AluOpType.mult

    nc = bacc.Bacc(
        "TRN2", target_bir_lowering=False, debug=False, num_devices=N_CORES
    )
    y8 = nc.dram_tensor("y8", [M8, C], f8, kind="ExternalInput").ap()
    y16 = nc.dram_tensor("y16", [M16, C], f16, kind="ExternalInput").ap()
    out = nc.dram_tensor(
        "out", [P, (M // P) * K_OUT], f16, kind="ExternalOutput"
    ).ap()

    KBMAX = 48
    T8_s = [nc.alloc_sbuf_tensor(f"T8b{i}", [P, KBMAX, C], f8) for i in range(BT)]
    T16_s = [
        nc.alloc_sbuf_tensor(f"T16b{i}", [P, FMAX, C], f16) for i in range(BT)
    ]
    E_s = [nc.alloc_sbuf_tensor(f"Eb{i}", [P, KBMAX, C], f16) for i in range(BE)]
    H_s = [nc.alloc_sbuf_tensor(f"Hb{i}", [P, KBMAX, C // 2], f16) for i in range(BH)]
    obuf = nc.alloc_sbuf_tensor("obuf", [P, M // P, K_OUT], f16)
    dum = nc.alloc_sbuf_tensor("dum", [P, 1], f16)

    jobs = list(JOBS)
    n = len(jobs)
    has_exp = [j["kb"] - j["f"] > 0 for j in jobs]
    has_ts = [j["f"] > 0 for j in jobs]
    nexp = np.cumsum(has_exp).tolist()

    # Emission-order cumulative counts. s_ts: +1 per ts. s_v: +1 per h1
    # (E-slot release marker). s_r: +1 per h2 (obuf written).
    ts_pos: dict[int, int] = {}
    h1_pos: dict[int, int] = {}
    cts = cv = cr = 0
    c_order = []
    for kind, i in VEC_PROG:
        if kind == "ts":
            assert has_ts[i]
            cts += 1
            ts_pos[i] = cts
        else:
            cv += 1
            h1_pos[i] = cv
            cr += 1
            c_order.append(i)
    assert cts == sum(has_ts) and cr == n

    cols = []
    col = 0
    for j in jobs:
        cols.append(col)
        col += j["kb"]
    assert col == M // P

    # Out-DMA split points keyed to h2 emission order [0,1,2,3,4,6,5].
    assert c_order == [0, 1, 2, 3, 4, 6, 5]
    SPLITS = [(4, 0, cols[4]), (5, cols[4], cols[5]), (7, cols[5], M // P)]

    with contextlib.ExitStack() as stack:
        block = stack.enter_context(nc.Block())
        dsem8 = [stack.enter_context(nc.semaphore(f"s_d8{i}")) for i in range(BT)]
        dsem16 = [stack.enter_context(nc.semaphore(f"s_d16{i}")) for i in range(BT)]
        s_exp = stack.enter_context(nc.semaphore("s_exp"))
        s_ts = stack.enter_context(nc.semaphore("s_ts"))
        s_v = stack.enter_context(nc.semaphore("s_v"))
        s_r = stack.enter_context(nc.semaphore("s_r"))
        s_out = stack.enter_context(nc.semaphore("s_out"))
        all_sems = dsem8 + dsem16 + [s_exp, s_ts, s_v, s_r, s_out]
        sem_nums = sorted(s.num for s in all_sems)

        pass8 = [0] * BT
        pass16 = [0] * BT
        d8_target = [0] * n
        d16_target = [0] * n
        for i, j in enumerate(jobs):
            if has_exp[i]:
                pass8[i % BT] += 1
                d8_target[i] = 16 * pass8[i % BT]
            if has_ts[i]:
                pass16[i % BT] += 1
                d16_target[i] = 16 * pass16[i % BT]

        @block.sync
        def _(sync):
            for i, j in enumerate(jobs):
                kb, f = j["kb"], j["f"]
                na = kb - f
                if i >= BT:
                    k = i - BT
                    if has_exp[k]:
                        sync.wait_ge(s_exp, nexp[k])
                    if has_ts[k]:
                        sync.wait_ge(s_ts, ts_pos[k])
                if na > 0:
                    ya = y8[j["base8"] : j["base8"] + P * na].rearrange(
                        "(p k) c -> p k c", p=P
                    )
                    sync.dma_start(
                        out=T8_s[i % BT].ap()[:, 0:na, :], in_=ya
                    ).then_inc(dsem8[i % BT], 16)
                if f > 0:
                    yb = y16[j["base16"] : j["base16"] + P * f].rearrange(
                        "(p k) c -> p k c", p=P
                    )
                    sync.dma_start(
                        out=T16_s[i % BT].ap()[:, 0:f, :], in_=yb
                    ).then_inc(dsem16[i % BT], 16)
            for rcnt, c0, c1 in SPLITS:
                sync.wait_ge(s_r, rcnt)
                sync.dma_start(
                    out=out[:, c0 * K_OUT : c1 * K_OUT],
                    in_=obuf.ap()[:, c0:c1, :],
                ).then_inc(s_out, 16)
            sync.wait_ge(s_out, 16 * len(SPLITS))
            sync.drain(semaphore_range=range(sem_nums[0], sem_nums[-1] + 1))
            sync.sem_clear(range(sem_nums[0], sem_nums[-1] + 1))

        @block.scalar
        def _(scalar):
            # Dummy 1-elem exp: forces ACT_TABLE_LOAD at body start,
            # overlapping the first input DMAs.
            scalar.activation(dum.ap()[:, 0:1], dum.ap()[:, 0:1], Exp)
            for i, j in enumerate(jobs):
                kb, f = j["kb"], j["f"]
                na = kb - f
                if na == 0:
                    continue
                scalar.wait_ge(dsem8[i % BT], d8_target[i])
                if i >= BE:
                    scalar.wait_ge(s_v, h1_pos[i - BE])
                scalar.activation(
                    E_s[i % BE].ap()[:, 0:na, :], T8_s[i % BT].ap()[:, 0:na, :], Exp
                ).then_inc(s_exp, 1)

        @block.vector
        def _(vec):
            for kind, i in VEC_PROG:
                j = jobs[i]
                kb, f = j["kb"], j["f"]
                E = E_s[i % BE].ap()
                H = H_s[i % BH].ap()
                if kind == "ts":
                    vec.wait_ge(dsem16[i % BT], d16_target[i])
                    if i >= BE:
                        assert h1_pos.get(i - BE) is not None  # emitted earlier
                    Ei = E.bitcast(i16)
                    vec.tensor_scalar(
                        Ei[:, kb - f : kb, :],
                        T16_s[i % BT].ap()[:, 0:f, :],
                        FE_A,
                        FE_B,
                        Mult,
                        Add,
                    ).then_inc(s_ts, 1)
                    continue
                if has_exp[i]:
                    vec.wait_ge(s_exp, nexp[i])
                vec.tensor_add(
                    H[:, 0:kb, :], E[:, 0:kb, 0 : C // 2], E[:, 0:kb, C // 2 : C]
                ).then_inc(s_v, 1)
                vec.tensor_add(
                    obuf.ap()[:, cols[i] : cols[i] + kb, :],
                    H[:, 0:kb, 0 : C // 4],
                    H[:, 0:kb, C // 4 : C // 2],
                ).then_inc(s_r, 1)

    nc.finalize()
    return nc


def _get_nc():
    if "nc" not in _CACHE:
        _CACHE["nc"] = _build_nc()
    return _CACHE["nc"]


def _make_in_maps(y_pred: np.ndarray):
    import ml_dtypes

    yp = np.asarray(y_pred)
    maps = []
    for c in range(N_CORES):
        yc = yp[c * M : (c + 1) * M]
        parts8, parts16 = [], []
        for j in JOBS:
            kb, f = j["kb"], j["f"]
            na = kb - f
            blk = yc[j["base"] : j["base"] + P * kb].reshape(P, kb, C)
            if na > 0:
                parts8.append(blk[:, 0:na, :].reshape(-1, C))
            if f > 0:
                parts16.append(blk[:, na:kb, :].reshape(-1, C))
        y8 = np.concatenate(parts8).astype(ml_dtypes.float8_e4m3)
        y16 = np.concatenate(parts16).astype(np.float16)
        maps.append(
            {"y8": np.ascontiguousarray(y8), "y16": np.ascontiguousarray(y16)}
        )
    return maps


def _run(in_maps, trace=False, **kwargs):
    from concourse.bass_utils import run_bass_kernel_spmd

    nc = _get_nc()
    return run_bass_kernel_spmd(
        nc, in_maps, list(range(N_CORES)), trace=trace, **kwargs
    )


def _combine(results, y_pred: np.ndarray, y_true: np.ndarray) -> np.ndarray:
    yp = np.asarray(y_pred)
    yt = np.asarray(y_true).reshape(-1).astype(np.int64)

    rowmap = np.empty((P, M // P), dtype=np.int64)
    col = 0
    for j in JOBS:
        kb = j["kb"]
        rowmap[:, col : col + kb] = (
            j["base"] + np.arange(P)[:, None] * kb + np.arange(kb)[None, :]
        )
        col += kb
    lse = np.empty(N, dtype=np.float64)
    for c in range(N_CORES):
        o = results[c]["out"].astype(np.float64).reshape(P, M // P, K_OUT)
        lse[c * M + rowmap.reshape(-1)] = np.log(o.sum(axis=2)).reshape(-1)

    picked = (
        np.take_along_axis(yp, yt[:, None], axis=1).reshape(-1).astype(np.float64)
    )
    ce = -(picked.sum() - lse.sum()) / N

    p1 = np.exp(yp[:, 0].astype(np.float64) - lse)
    lp = np.log(p1 + EPS)
    lq = np.log((1.0 + EPS) - p1)
    nj = np.bincount(yt, minlength=C).astype(np.float64)
    s = BETA * (1.0 - nj / (N - nj[0]))
    v = np.where(yt == 0, ALPHA * lp, s[yt] * lq)
    loss = ce - v.sum() / N
    return np.asarray(loss, dtype=np.float32)


def kernel(y_pred: np.ndarray, y_true: np.ndarray) -> np.ndarray:
    in_maps = _make_in_maps(y_pred)
    res = _run(in_maps, trace=False)
    return _combine(res.results, y_pred, y_true)


# revision 12
# speedup vs baseline: 1.5049x; 1.0018x over previous
"""Trainium2 Bass kernel for nn_CustomLoss_19061064859882.

loss = CE(y_pred, y_true) - penalty/N, where the penalty uses
p1 = softmax(y_pred)[:, 0] and per-class weights from the label histogram.

Device computes per-row partial sumexp over the 128 classes (data-parallel
over the 8 cores): each row's 128 exps are reduced on-device to 32 partial
sums (h1+h2 pairwise-add chain); the host finishes the remaining adds and
the log in float64, plus the other O(N) bookkeeping (picked logits, label
bincount, final scalar).

Wire format: rows destined for the ACT engine travel as fp8 e4m3 (costs
~1e-3 in log-sum accuracy -- measured std 0.008, bias -8e-5 -- far inside
the 2e-2 tolerance), halving input DMA for those rows. Rows destined for
the DVE fast-exp stay fp16 (the 4x-mode tensor_scalar needs 16-bit).

Two-engine compute (GpSimd deliberately idle: the POOL slot shares the
SBUF read port with the DVE, so concurrent GpSimd tensor ops degrade DVE
2-port instructions ~3.4x, measured):
  - ACT (scalar): native EXP, fp8 in / fp16 out, 1 elem/cycle/lane.
  - DVE (vector): Schraudolph fast-exp in ONE 4x-mode tensor_scalar op
    (i16 = int16(x*1477.32 + 15301.3), bit pattern read back as fp16), plus
    the h1+h2 halving chain, h2 writing 32 fp16 partials per row straight
    into the output buffer.
The first job is kb=32 (one sizable DMA amortizes the cold-ring latency),
the last jobs are small to shorten the post-EXP chain tail, and the
out-DMA is split in three so all but the last ~0.25 MiB overlaps compute.
"""

import sys

import numpy as np

if "/opt/trn_rl_repo" not in sys.path:
    sys.path.insert(0, "/opt/trn_rl_repo")

N_CORES = 8
N = 262144
C = 128  # classes
M = N // N_CORES  # rows per core (32768)
P = 128  # SBUF partitions
K_OUT = 32  # partial sums kept per row (device reduces 128 -> 32)
ALPHA = 0.5
BETA = 0.5
EPS = 1e-9

# Schraudolph fast-exp constants (fp16 bit trick), calibrated so the MEAN of
# log(row_sum_approx) - lse is ~0 on N(0,1) logits.
FE_A = 1477.3197218702985  # 1024 * log2(e)
FE_B = 15301.33  # 1024 * (15 - delta), delta ~= 0.0573

# Jobs: kb = rows per partition, f = rows via DVE fast-exp (rest on ACT).
# f is graded DOWN across jobs: late jobs are ACT-heavy so the trailing
# chain work on DVE stays covered by remaining EXP work (suffix balance).
JOBS = [
    dict(kb=32, f=10),
    dict(kb=48, f=24),
    dict(kb=48, f=20),
    dict(kb=48, f=18),
    dict(kb=48, f=14),
    dict(kb=24, f=8),
    dict(kb=8, f=0),  # tail job, all-ACT (tiny last chain)
]
_base = _b8 = _b16 = 0
for _j in JOBS:
    _j["base"] = _base
    _j["base8"] = _b8
    _j["base16"] = _b16
    _base += P * _j["kb"]
    _b8 += P * (_j["kb"] - _j["f"])
    _b16 += P * _j["f"]
assert _base == M
M8, M16 = _b8, _b16

BT = 4  # T8/T16 (input) buffer slots
BE = 4  # E (exp) buffer slots
BH = 3  # H chain buffer slots
FMAX = 24

# Vector emission order (op level): per-job [ts, chain]; the last two ts
# ops are hoisted ahead of chain 4 so the final stretch is chains only.
VEC_PROG = [
    ("ts", 0),
    ("c", 0),
    ("ts", 1),
    ("c", 1),
    ("ts", 2),
    ("c", 2),
    ("ts", 3),
    ("c", 3),
    ("ts", 4),
    ("ts", 5),
    ("c", 4),
    ("c", 5),
    ("c", 6),
]

_CACHE: dict = {}


def _build_nc():
    import contextlib

    import concourse.bacc as bacc
    import concourse.mybir as mybir

    f16 = mybir.dt.float16
    f8 = mybir.dt.float8e4
    i16 = mybir.dt.int16
    Exp = mybir.ActivationFunctionType.Exp
    Add = mybir.AluOpType.add
    Mult = mybir.AluOpType.mult

    nc = bacc.Bacc(
        "TRN2", target_bir_lowering=False, debug=False, num_devices=N_CORES
    )
    y8 = nc.dram_tensor("y8", [M8, C], f8, kind="ExternalInput").ap()
    y16 = nc.dram_tensor("y16", [M16, C], f16, kind="ExternalInput").ap()
    out = nc.dram_tensor(
        "out", [P, (M // P) * K_OUT], f16, kind="ExternalOutput"
    ).ap()

    KBMAX = 48
    T8_s = [nc.alloc_sbuf_tensor(f"T8b{i}", [P, KBMAX, C], f8) for i in range(BT)]
    T16_s = [
        nc.alloc_sbuf_tensor(f"T16b{i}", [P, FMAX, C], f16) for i in range(BT)
    ]
    E_s = [nc.alloc_sbuf_tensor(f"Eb{i}", [P, KBMAX, C], f16) for i in range(BE)]
    H_s = [nc.alloc_sbuf_tensor(f"Hb{i}", [P, KBMAX, C // 2], f16) for i in range(BH)]
    obuf = nc.alloc_sbuf_tensor("obuf", [P, M // P, K_OUT], f16)
    dum = nc.alloc_sbuf_tensor("dum", [P, 1], f16)

    jobs = list(JOBS)
    n = len(jobs)
    has_exp = [j["kb"] - j["f"] > 0 for j in jobs]
    has_ts = [j["f"] > 0 for j in jobs]
    nexp = np.cumsum(has_exp).tolist()

    # Emission-order cumulative counts. s_ts: +1 per ts. s_v: +1 per h1
    # (E-slot release marker). s_r: +1 per h2 (obuf written).
    ts_pos: dict[int, int] = {}
    h1_pos: dict[int, int] = {}
    cts = cv = cr = 0
    c_order = []
    for kind, i in VEC_PROG:
        if kind == "ts":
            assert has_ts[i]
            cts += 1
            ts_pos[i] = cts
        else:
            cv += 1
            h1_pos[i] = cv
            cr += 1
            c_order.append(i)
    assert cts == sum(has_ts) and cr == n

    cols = []
    col = 0
    for j in jobs:
        cols.append(col)
        col += j["kb"]
    assert col == M // P

    # Out-DMA split points keyed to h2 emission order (in job order here);
    # the final split is one tiny 8-column DMA so almost nothing waits on
    # the last chain.
    assert c_order == [0, 1, 2, 3, 4, 5, 6]
    SPLITS = [
        (4, 0, cols[4]),
        (5, cols[4], cols[5]),
        (6, cols[5], cols[6]),
        (7, cols[6], M // P),
    ]

    with contextlib.ExitStack() as stack:
        block = stack.enter_context(nc.Block())
        dsem8 = [stack.enter_context(nc.semaphore(f"s_d8{i}")) for i in range(BT)]
        dsem16 = [stack.enter_context(nc.semaphore(f"s_d16{i}")) for i in range(BT)]
        s_exp = stack.enter_context(nc.semaphore("s_exp"))
        s_ts = stack.enter_context(nc.semaphore("s_ts"))
        s_v = stack.enter_context(nc.semaphore("s_v"))
        s_r = stack.enter_context(nc.semaphore("s_r"))
        s_out = stack.enter_context(nc.semaphore("s_out"))
        all_sems = dsem8 + dsem16 + [s_exp, s_ts, s_v, s_r, s_out]
        sem_nums = sorted(s.num for s in all_sems)

        pass8 = [0] * BT
        pass16 = [0] * BT
        d8_target = [0] * n
        d16_target = [0] * n
        for i, j in enumerate(jobs):
            if has_exp[i]:
                pass8[i % BT] += 1
                d8_target[i] = 16 * pass8[i % BT]
            if has_ts[i]:
                pass16[i % BT] += 1
                d16_target[i] = 16 * pass16[i % BT]

        @block.sync
        def _(sync):
            for i, j in enumerate(jobs):
                kb, f = j["kb"], j["f"]
                na = kb - f
                if i >= BT:
                    k = i - BT
                    if has_exp[k]:
                        sync.wait_ge(s_exp, nexp[k])
                    if has_ts[k]:
                        sync.wait_ge(s_ts, ts_pos[k])
                if na > 0:
                    ya = y8[j["base8"] : j["base8"] + P * na].rearrange(
                        "(p k) c -> p k c", p=P
                    )
                    sync.dma_start(
                        out=T8_s[i % BT].ap()[:, 0:na, :], in_=ya
                    ).then_inc(dsem8[i % BT], 16)
                if f > 0:
                    yb = y16[j["base16"] : j["base16"] + P * f].rearrange(
                        "(p k) c -> p k c", p=P
                    )
                    sync.dma_start(
                        out=T16_s[i % BT].ap()[:, 0:f, :], in_=yb
                    ).then_inc(dsem16[i % BT], 16)
            for rcnt, c0, c1 in SPLITS:
                sync.wait_ge(s_r, rcnt)
                sync.dma_start(
                    out=out[:, c0 * K_OUT : c1 * K_OUT],
                    in_=obuf.ap()[:, c0:c1, :],
                ).then_inc(s_out, 16)
            sync.wait_ge(s_out, 16 * len(SPLITS))
            sync.drain(semaphore_range=range(sem_nums[0], sem_nums[-1] + 1))
            sync.sem_clear(range(sem_nums[0], sem_nums[-1] + 1))

        @block.scalar
        def _(scalar):
            # Dummy 1-elem exp: forces ACT_TABLE_LOAD at body start,
            # overlapping the first input DMAs.
            scalar.activation(dum.ap()[:, 0:1], dum.ap()[:, 0:1], Exp)
            for i, j in enumerate(jobs):
                kb, f = j["kb"], j["f"]
                na = kb - f
                if na == 0:
                    continue
                scalar.wait_ge(dsem8[i % BT], d8_target[i])
                if i >= BE:
                    scalar.wait_ge(s_v, h1_pos[i - BE])
                scalar.activation(
                    E_s[i % BE].ap()[:, 0:na, :], T8_s[i % BT].ap()[:, 0:na, :], Exp
                ).then_inc(s_exp, 1)

        @block.vector
        def _(vec):
            for kind, i in VEC_PROG:
                j = jobs[i]
                kb, f = j["kb"], j["f"]
                E = E_s[i % BE].ap()
                H = H_s[i % BH].ap()
                if kind == "ts":
                    vec.wait_ge(dsem16[i % BT], d16_target[i])
                    if i >= BE:
                        assert h1_pos.get(i - BE) is not None  # emitted earlier
                    Ei = E.bitcast(i16)
                    vec.tensor_scalar(
                        Ei[:, kb - f : kb, :],
                        T16_s[i % BT].ap()[:, 0:f, :],
                        FE_A,
                        FE_B,
                        Mult,
                        Add,
                    ).then_inc(s_ts, 1)
                    continue
                if has_exp[i]:
                    vec.wait_ge(s_exp, nexp[i])
                vec.tensor_add(
                    H[:, 0:kb, :], E[:, 0:kb, 0 : C // 2], E[:, 0:kb, C // 2 : C]
                ).then_inc(s_v, 1)
                vec.tensor_add(
                    obuf.ap()[:, cols[i] : cols[i] + kb, :],
                    H[:, 0:kb, 0 : C // 4],
                    H[:, 0:kb, C // 4 : C // 2],
                ).then_inc(s_r, 1)

    nc.finalize()
    return nc


def _get_nc():
    if "nc" not in _CACHE:
        _CACHE["nc"] = _build_nc()
    return _CACHE["nc"]


def _make_in_maps(y_pred: np.ndarray):
    import ml_dtypes

    yp = np.asarray(y_pred)
    maps = []
    for c in range(N_CORES):
        yc = yp[c * M : (c + 1) * M]
        parts8, parts16 = [], []
        for j in JOBS:
            kb, f = j["kb"], j["f"]
            na = kb - f
            blk = yc[j["base"] : j["base"] + P * kb].reshape(P, kb, C)
            if na > 0:
                parts8.append(blk[:, 0:na, :].reshape(-1, C))
            if f > 0:
                parts16.append(blk[:, na:kb, :].reshape(-1, C))
        y8 = np.concatenate(parts8).astype(ml_dtypes.float8_e4m3)
        y16 = np.concatenate(parts16).astype(np.float16)
        maps.append(
            {"y8": np.ascontiguousarray(y8), "y16": np.ascontiguousarray(y16)}
        )
    return maps


def _run(in_maps, trace=False, **kwargs):
    from concourse.bass_utils import run_bass_kernel_spmd

    nc = _get_nc()
    return run_bass_kernel_spmd(
        nc, in_maps, list(range(N_CORES)), trace=trace, **kwargs
    )


def _combine(results, y_pred: np.ndarray, y_true: np.ndarray) -> np.ndarray:
    yp = np.asarray(y_pred)
    yt = np.asarray(y_true).reshape(-1).astype(np.int64)

    rowmap = np.empty((P, M // P), dtype=np.int64)
    col = 0
    for j in JOBS:
        kb = j["kb"]
        rowmap[:, col : col + kb] = (
            j["base"] + np.arange(P)[:, None] * kb + np.arange(kb)[None, :]
        )
        col += kb
    lse = np.empty(N, dtype=np.float64)
    for c in range(N_CORES):
        o = results[c]["out"].astype(np.float64).reshape(P, M // P, K_OUT)
        lse[c * M + rowmap.reshape(-1)] = np.log(o.sum(axis=2)).reshape(-1)

    picked = (
        np.take_along_axis(yp, yt[:, None], axis=1).reshape(-1).astype(np.float64)
    )
    ce = -(picked.sum() - lse.sum()) / N

    p1 = np.exp(yp[:, 0].astype(np.float64) - lse)
    lp = np.log(p1 + EPS)
    lq = np.log((1.0 + EPS) - p1)
    nj = np.bincount(yt, minlength=C).astype(np.float64)
    s = BETA * (1.0 - nj / (N - nj[0]))
    v = np.where(yt == 0, ALPHA * lp, s[yt] * lq)
    loss = ce - v.sum() / N
    return np.asarray(loss, dtype=np.float32)


def kernel(y_pred: np.ndarray, y_true: np.ndarray) -> np.ndarray:
    in_maps = _make_in_maps(y_pred)
    res = _run(in_maps, trace=False)
    return _combine(res.results, y_pred, y_true)


# revision 13
# speedup vs baseline: 1.5081x; 1.0021x over previous
"""Trainium2 Bass kernel for nn_CustomLoss_19061064859882.

loss = CE(y_pred, y_true) - penalty/N, where the penalty uses
p1 = softmax(y_pred)[:, 0] and per-class weights from the label histogram.

Device computes per-row partial sumexp over the 128 classes (data-parallel
over the 8 cores): each row's 128 exps are reduced on-device to 32 partial
sums (h1+h2 pairwise-add chain); the host finishes the remaining adds and
the log in float64, plus the other O(N) bookkeeping (picked logits, label
bincount, final scalar).

Wire format: rows destined for the ACT engine travel as fp8 e4m3 (costs
~1e-3 in log-sum accuracy -- measured std 0.008, bias -8e-5 -- far inside
the 2e-2 tolerance), halving input DMA for those rows. Rows destined for
the DVE fast-exp stay fp16 (the 4x-mode tensor_scalar needs 16-bit).

Two-engine compute (GpSimd deliberately idle: the POOL slot shares the
SBUF read port with the DVE, so concurrent GpSimd tensor ops degrade DVE
2-port instructions ~3.4x, measured):
  - ACT (scalar): native EXP, fp8 in / fp16 out, 1 elem/cycle/lane.
  - DVE (vector): Schraudolph fast-exp in ONE 4x-mode tensor_scalar op
    (i16 = int16(x*1477.32 + 15301.3), bit pattern read back as fp16), plus
    the h1+h2 halving chain, h2 writing 32 fp16 partials per row straight
    into the output buffer.
The first job is kb=32 (one sizable DMA amortizes the cold-ring latency),
the last jobs are small to shorten the post-EXP chain tail, and the
out-DMA is split in three so all but the last ~0.25 MiB overlaps compute.
"""

import sys

import numpy as np

if "/opt/trn_rl_repo" not in sys.path:
    sys.path.insert(0, "/opt/trn_rl_repo")

N_CORES = 8
N = 262144
C = 128  # classes
M = N // N_CORES  # rows per core (32768)
P = 128  # SBUF partitions
K_OUT = 32  # partial sums kept per row (device reduces 128 -> 32)
ALPHA = 0.5
BETA = 0.5
EPS = 1e-9

# Schraudolph fast-exp constants (fp16 bit trick), calibrated so the MEAN of
# log(row_sum_approx) - lse is ~0 on N(0,1) logits.
FE_A = 1477.3197218702985  # 1024 * log2(e)
FE_B = 15301.33  # 1024 * (15 - delta), delta ~= 0.0573

# Jobs: kb = rows per partition, f = rows via DVE fast-exp (rest on ACT).
# f is graded DOWN across jobs: late jobs are ACT-heavy so the trailing
# chain work on DVE stays covered by remaining EXP work (suffix balance).
JOBS = [
    dict(kb=32, f=10),
    dict(kb=48, f=24),
    dict(kb=48, f=20),
    dict(kb=48, f=18),
    dict(kb=48, f=14),
    dict(kb=24, f=8),
    dict(kb=8, f=0),  # tail job, all-ACT (tiny last chain)
]
_base = _b8 = _b16 = 0
for _j in JOBS:
    _j["base"] = _base
    _j["base8"] = _b8
    _j["base16"] = _b16
    _base += P * _j["kb"]
    _b8 += P * (_j["kb"] - _j["f"])
    _b16 += P * _j["f"]
assert _base == M
M8, M16 = _b8, _b16

BT = 4  # T8/T16 (input) buffer slots
BE = 4  # E (exp) buffer slots
BH = 3  # H chain buffer slots
FMAX = 24

# Vector emission order (op level): per-job [ts, chain]; the last two ts
# ops are hoisted ahead of chain 4 so the final stretch is chains only.
VEC_PROG = [
    ("ts", 0),
    ("c", 0),
    ("ts", 1),
    ("c", 1),
    ("ts", 2),
    ("c", 2),
    ("ts", 3),
    ("c", 3),
    ("ts", 4),
    ("ts", 5),
    ("c", 4),
    ("c", 5),
    ("c", 6),
]

_CACHE: dict = {}


def _build_nc():
    import contextlib

    import concourse.bacc as bacc
    import concourse.mybir as mybir

    f16 = mybir.dt.float16
    f8 = mybir.dt.float8e4
    i16 = mybir.dt.int16
    Exp = mybir.ActivationFunctionType.Exp
    Add = mybir.AluOpType.add
    Mult = mybir.AluOpType.mult

    nc = bacc.Bacc(
        "TRN2", target_bir_lowering=False, debug=False, num_devices=N_CORES
    )
    y8 = nc.dram_tensor("y8", [M8, C], f8, kind="ExternalInput").ap()
    y16 = nc.dram_tensor("y16", [M16, C], f16, kind="ExternalInput").ap()
    out = nc.dram_tensor(
        "out", [P, (M // P) * K_OUT], f16, kind="ExternalOutput"
    ).ap()

    KBMAX = 48
    T8_s = [nc.alloc_sbuf_tensor(f"T8b{i}", [P, KBMAX, C], f8) for i in range(BT)]
    T16_s = [
        nc.alloc_sbuf_tensor(f"T16b{i}", [P, FMAX, C], f16) for i in range(BT)
    ]
    E_s = [nc.alloc_sbuf_tensor(f"Eb{i}", [P, KBMAX, C], f16) for i in range(BE)]
    H_s = [nc.alloc_sbuf_tensor(f"Hb{i}", [P, KBMAX, C // 2], f16) for i in range(BH)]
    obuf = nc.alloc_sbuf_tensor("obuf", [P, M // P, K_OUT], f16)
    dum = nc.alloc_sbuf_tensor("dum", [P, 1], f16)

    jobs = list(JOBS)
    n = len(jobs)
    has_exp = [j["kb"] - j["f"] > 0 for j in jobs]
    has_ts = [j["f"] > 0 for j in jobs]
    nexp = np.cumsum(has_exp).tolist()

    # Emission-order cumulative counts. s_ts: +1 per ts. s_v: +1 per h1
    # (E-slot release marker). s_r: +1 per h2 (obuf written).
    ts_pos: dict[int, int] = {}
    h1_pos: dict[int, int] = {}
    cts = cv = cr = 0
    c_order = []
    for kind, i in VEC_PROG:
        if kind == "ts":
            assert has_ts[i]
            cts += 1
            ts_pos[i] = cts
        else:
            cv += 1
            h1_pos[i] = cv
            cr += 1
            c_order.append(i)
    assert cts == sum(has_ts) and cr == n

    cols = []
    col = 0
    for j in jobs:
        cols.append(col)
        col += j["kb"]
    assert col == M // P

    # Out-DMA split points keyed to h2 emission order (in job order here);
    # the final split is one tiny 8-column DMA so almost nothing waits on
    # the last chain.
    assert c_order == [0, 1, 2, 3, 4, 5, 6]
    SPLITS = [
        (4, 0, cols[4]),
        (5, cols[4], cols[5]),
        (6, cols[5], cols[6]),
        (7, cols[6], M // P),
    ]

    with contextlib.ExitStack() as stack:
        block = stack.enter_context(nc.Block())
        dsem8 = [stack.enter_context(nc.semaphore(f"s_d8{i}")) for i in range(BT)]
        dsem16 = [stack.enter_context(nc.semaphore(f"s_d16{i}")) for i in range(BT)]
        s_exp = stack.enter_context(nc.semaphore("s_exp"))
        s_ts = stack.enter_context(nc.semaphore("s_ts"))
        s_v = stack.enter_context(nc.semaphore("s_v"))
        s_r = stack.enter_context(nc.semaphore("s_r"))
        s_out = stack.enter_context(nc.semaphore("s_out"))
        all_sems = dsem8 + dsem16 + [s_exp, s_ts, s_v, s_r, s_out]
        sem_nums = sorted(s.num for s in all_sems)

        pass8 = [0] * BT
        pass16 = [0] * BT
        d8_target = [0] * n
        d16_target = [0] * n
        for i, j in enumerate(jobs):
            if has_exp[i]:
                pass8[i % BT] += 1
                d8_target[i] = 16 * pass8[i % BT]
            if has_ts[i]:
                pass16[i % BT] += 1
                d16_target[i] = 16 * pass16[i % BT]

        # Input issue order: each job's fp8 (ACT) part is prioritized one
        # job ahead of the fp16 (DVE) parts -- the EXP stream is the pacer,
        # and the sync HWDGE queue drains strictly in issue order.
        issue_order = []
        for i in range(n):
            if has_exp[i]:
                issue_order.append(("d8", i))
            if i >= 1 and has_ts[i - 1]:
                issue_order.append(("d16", i - 1))
        if has_ts[n - 1]:
            issue_order.append(("d16", n - 1))

        @block.sync
        def _(sync):
            for kind, i in issue_order:
                j = jobs[i]
                kb, f = j["kb"], j["f"]
                na = kb - f
                if kind == "d8":
                    if i >= BT and has_exp[i - BT]:
                        sync.wait_ge(s_exp, nexp[i - BT])
                    ya = y8[j["base8"] : j["base8"] + P * na].rearrange(
                        "(p k) c -> p k c", p=P
                    )
                    sync.dma_start(
                        out=T8_s[i % BT].ap()[:, 0:na, :], in_=ya
                    ).then_inc(dsem8[i % BT], 16)
                else:
                    if i >= BT and has_ts[i - BT]:
                        sync.wait_ge(s_ts, ts_pos[i - BT])
                    yb = y16[j["base16"] : j["base16"] + P * f].rearrange(
                        "(p k) c -> p k c", p=P
                    )
                    sync.dma_start(
                        out=T16_s[i % BT].ap()[:, 0:f, :], in_=yb
                    ).then_inc(dsem16[i % BT], 16)
            for rcnt, c0, c1 in SPLITS:
                sync.wait_ge(s_r, rcnt)
                sync.dma_start(
                    out=out[:, c0 * K_OUT : c1 * K_OUT],
                    in_=obuf.ap()[:, c0:c1, :],
                ).then_inc(s_out, 16)
            sync.wait_ge(s_out, 16 * len(SPLITS))
            sync.drain(semaphore_range=range(sem_nums[0], sem_nums[-1] + 1))
            sync.sem_clear(range(sem_nums[0], sem_nums[-1] + 1))

        @block.scalar
        def _(scalar):
            # Dummy 1-elem exp: forces ACT_TABLE_LOAD at body start,
            # overlapping the first input DMAs.
            scalar.activation(dum.ap()[:, 0:1], dum.ap()[:, 0:1], Exp)
            for i, j in enumerate(jobs):
                kb, f = j["kb"], j["f"]
                na = kb - f
                if na == 0:
                    continue
                scalar.wait_ge(dsem8[i % BT], d8_target[i])
                if i >= BE:
                    scalar.wait_ge(s_v, h1_pos[i - BE])
                scalar.activation(
                    E_s[i % BE].ap()[:, 0:na, :], T8_s[i % BT].ap()[:, 0:na, :], Exp
                ).then_inc(s_exp, 1)

        @block.vector
        def _(vec):
            for kind, i in VEC_PROG:
                j = jobs[i]
                kb, f = j["kb"], j["f"]
                E = E_s[i % BE].ap()
                H = H_s[i % BH].ap()
                if kind == "ts":
                    vec.wait_ge(dsem16[i % BT], d16_target[i])
                    if i >= BE:
                        assert h1_pos.get(i - BE) is not None  # emitted earlier
                    Ei = E.bitcast(i16)
                    vec.tensor_scalar(
                        Ei[:, kb - f : kb, :],
                        T16_s[i % BT].ap()[:, 0:f, :],
                        FE_A,
                        FE_B,
                        Mult,
                        Add,
                    ).then_inc(s_ts, 1)
                    continue
                if has_exp[i]:
                    vec.wait_ge(s_exp, nexp[i])
                vec.tensor_add(
                    H[:, 0:kb, :], E[:, 0:kb, 0 : C // 2], E[:, 0:kb, C // 2 : C]
                ).then_inc(s_v, 1)
                vec.tensor_add(
                    obuf.ap()[:, cols[i] : cols[i] + kb, :],
                    H[:, 0:kb, 0 : C // 4],
                    H[:, 0:kb, C // 4 : C // 2],
                ).then_inc(s_r, 1)

    nc.finalize()
    return nc


def _get_nc():
    if "nc" not in _CACHE:
        _CACHE["nc"] = _build_nc()
    return _CACHE["nc"]


def _make_in_maps(y_pred: np.ndarray):
    import ml_dtypes

    yp = np.asarray(y_pred)
    maps = []
    for c in range(N_CORES):
        yc = yp[c * M : (c + 1) * M]
        parts8, parts16 = [], []
        for j in JOBS:
            kb, f = j["kb"], j["f"]
            na = kb - f
            blk = yc[j["base"] : j["base"] + P * kb].reshape(P, kb, C)
            if na > 0:
                parts8.append(blk[:, 0:na, :].reshape(-1, C))
            if f > 0:
                parts16.append(blk[:, na:kb, :].reshape(-1, C))
        y8 = np.concatenate(parts8).astype(ml_dtypes.float8_e4m3)
        y16 = np.concatenate(parts16).astype(np.float16)
        maps.append(
            {"y8": np.ascontiguousarray(y8), "y16": np.ascontiguousarray(y16)}
        )
    return maps


def _run(in_maps, trace=False, **kwargs):
    from concourse.bass_utils import run_bass_kernel_spmd

    nc = _get_nc()
    return run_bass_kernel_spmd(
        nc, in_maps, list(range(N_CORES)), trace=trace, **kwargs
    )


def _combine(results, y_pred: np.ndarray, y_true: np.ndarray) -> np.ndarray:
    yp = np.asarray(y_pred)
    yt = np.asarray(y_true).reshape(-1).astype(np.int64)

    rowmap = np.empty((P, M // P), dtype=np.int64)
    col = 0
    for j in JOBS:
        kb = j["kb"]
        rowmap[:, col : col + kb] = (
            j["base"] + np.arange(P)[:, None] * kb + np.arange(kb)[None, :]
        )
        col += kb
    lse = np.empty(N, dtype=np.float64)
    for c in range(N_CORES):
        o = results[c]["out"].astype(np.float64).reshape(P, M // P, K_OUT)
        lse[c * M + rowmap.reshape(-1)] = np.log(o.sum(axis=2)).reshape(-1)

    picked = (
        np.take_along_axis(yp, yt[:, None], axis=1).reshape(-1).astype(np.float64)
    )
    ce = -(picked.sum() - lse.sum()) / N

    p1 = np.exp(yp[:, 0].astype(np.float64) - lse)
    lp = np.log(p1 + EPS)
    lq = np.log((1.0 + EPS) - p1)
    nj = np.bincount(yt, minlength=C).astype(np.float64)
    s = BETA * (1.0 - nj / (N - nj[0]))
    v = np.where(yt == 0, ALPHA * lp, s[yt] * lq)
    loss = ce - v.sum() / N
    return np.asarray(loss, dtype=np.float32)


def kernel(y_pred: np.ndarray, y_true: np.ndarray) -> np.ndarray:
    in_maps = _make_in_maps(y_pred)
    res = _run(in_maps, trace=False)
    return _combine(res.results, y_pred, y_true)
